# revision 4
# baseline (speedup 1.0000x reference)
"""Trainium2 Bass kernel for nn_Attention_80805514707533.

Recurrent attention scan: B=512, T=512, C=64, H=128.
Sharding: H across 8 cores (16 heads each); full batch B=512 rides the
matmul moving dimension. C=64 lives on partitions; heads are packed in
pairs (2 x 64 = 128 partitions) with block-diagonal stationary weights.

Per step t (per core, j = head-pair 0..7):
  pre[j]  = Wi_cat[j].T @ xT_t  +  Wa_blk[j].T @ att[:,j,:]      (PSUM)
  v[j]    = tanh(pre[j])                                          (ACT)
  e[j]    = We_blk[j].T @ v[j]                                    (PSUM)
  u[j]    = exp(e[j])                                             (ACT)
  w       = u * xdup  (broadcast over j)                          (GPSIMD)
  S       = sum_c u   via ones16 matmuls  -> [16, B]              (PE)
  num     = sum_c u*x via ones16 matmuls  -> [16, B]              (PE)
  rS      = 1/S                                                   (DVE)
  out_t   = num * rS  -> DMA to DRAM                              (DVE)
  att'    = u * bcast(rS)   (bcast via selector matmul)           (PE+DVE)
"""

import os
import numpy as np

B, T, C, H = 512, 512, 64, 128
NCORES = 8
HL = H // NCORES          # heads per core = 16
NPAIR = HL // 2           # head pairs per core = 8
FP = None                 # set lazily (mybir.dt.float32)


def _build_nc(t_steps: int):
    import concourse.bass as bass
    import concourse.bacc as bacc
    import concourse.mybir as mybir
    import concourse.tile as tile
    from contextlib import ExitStack

    fp32 = mybir.dt.float32
    nc = bacc.Bacc("TRN2", target_bir_lowering=False, debug=False,
                   num_devices=NCORES)

    xT_d = nc.dram_tensor("xT", [C, t_steps, B], fp32, kind="ExternalInput")
    wi_d = nc.dram_tensor("wi", [C, NPAIR, 128], fp32, kind="ExternalInput")
    wa_d = nc.dram_tensor("wa", [128, NPAIR, 128], fp32, kind="ExternalInput")
    we_d = nc.dram_tensor("we", [128, NPAIR, 128], fp32, kind="ExternalInput")
    on_d = nc.dram_tensor("ones16", [128, NPAIR, HL], fp32, kind="ExternalInput")
    sel_d = nc.dram_tensor("sel", [HL, NPAIR, 128], fp32, kind="ExternalInput")
    out_d = nc.dram_tensor("out", [t_steps, HL, B], fp32, kind="ExternalOutput")

    with ExitStack() as ctx:
        tc = ctx.enter_context(tile.TileContext(nc))
        singles = ctx.enter_context(tc.tile_pool(name="singles", bufs=1))
        state = ctx.enter_context(tc.tile_pool(name="state", bufs=1))
        xpool = ctx.enter_context(tc.tile_pool(name="xpool", bufs=3))
        vpool = ctx.enter_context(tc.tile_pool(name="vpool", bufs=2))
        upool = ctx.enter_context(tc.tile_pool(name="upool", bufs=2))
        wpool = ctx.enter_context(tc.tile_pool(name="wpool", bufs=2))
        spool = ctx.enter_context(tc.tile_pool(name="spool", bufs=3))
        opool = ctx.enter_context(tc.tile_pool(name="opool", bufs=3))
        ps_pre = ctx.enter_context(tc.tile_pool(name="ps_pre", bufs=2, space="PSUM"))
        ps_e = ctx.enter_context(tc.tile_pool(name="ps_e", bufs=2, space="PSUM"))
        ps_sn = ctx.enter_context(tc.tile_pool(name="ps_sn", bufs=1, space="PSUM"))
        ps_bc = ctx.enter_context(tc.tile_pool(name="ps_bc", bufs=2, space="PSUM"))

        wi_sb = singles.tile([C, NPAIR, 128], fp32)
        wa_sb = singles.tile([128, NPAIR, 128], fp32)
        we_sb = singles.tile([128, NPAIR, 128], fp32)
        on_sb = singles.tile([128, NPAIR, HL], fp32)
        sel_sb = singles.tile([HL, NPAIR, 128], fp32)
        nc.sync.dma_start(out=wi_sb, in_=wi_d[:])
        nc.sync.dma_start(out=wa_sb, in_=wa_d[:])
        nc.sync.dma_start(out=we_sb, in_=we_d[:])
        nc.sync.dma_start(out=on_sb, in_=on_d[:])
        nc.sync.dma_start(out=sel_sb, in_=sel_d[:])

        att = state.tile([128, NPAIR, B], fp32)
        nc.vector.memset(att, 1.0 / C)

        for t in range(t_steps):
            xdup = xpool.tile([128, B], fp32)
            nc.sync.dma_start(out=xdup[0:C, :], in_=xT_d[:, t, :])
            nc.sync.dma_start(out=xdup[C:128, :], in_=xT_d[:, t, :])

            v_sb = vpool.tile([128, NPAIR, B], fp32)
            for j in range(NPAIR):
                pre = ps_pre.tile([128, B], fp32)
                nc.tensor.matmul(pre, wi_sb[:, j, :], xdup[0:C, :],
                                 start=True, stop=False)
                nc.tensor.matmul(pre, wa_sb[:, j, :], att[:, j, :],
                                 start=False, stop=True)
                nc.scalar.activation(v_sb[:, j, :], pre,
                                     mybir.ActivationFunctionType.Tanh)

            u_sb = upool.tile([128, NPAIR, B], fp32)
            for j in range(NPAIR):
                e = ps_e.tile([128, B], fp32)
                nc.tensor.matmul(e, we_sb[:, j, :], v_sb[:, j, :],
                                 start=True, stop=True)
                nc.scalar.activation(u_sb[:, j, :], e,
                                     mybir.ActivationFunctionType.Exp)

            # w = u * x  (x broadcast over pairs dim) on GPSIMD
            w_sb = wpool.tile([128, NPAIR, B], fp32)
            xb = bass.AP(xdup.tensor, xdup.offset,
                         [xdup.ap[0], [0, NPAIR], xdup.ap[1]])
            nc.gpsimd.tensor_mul(w_sb, u_sb, xb)

            S_ps = ps_sn.tile([HL, B], fp32)
            num_ps = ps_sn.tile([HL, B], fp32)
            for j in range(NPAIR):
                nc.tensor.matmul(S_ps, on_sb[:, j, :], u_sb[:, j, :],
                                 start=(j == 0), stop=(j == NPAIR - 1))
            for j in range(NPAIR):
                nc.tensor.matmul(num_ps, on_sb[:, j, :], w_sb[:, j, :],
                                 start=(j == 0), stop=(j == NPAIR - 1))

            rS = spool.tile([HL, B], fp32)
            nc.vector.reciprocal(rS, S_ps)
            outb = opool.tile([HL, B], fp32)
            nc.vector.tensor_mul(outb, num_ps, rS)
            nc.sync.dma_start(out=out_d[t], in_=outb)

            # att' = u * bcast(rS) ; bcast via selector matmul per pair
            for j in range(NPAIR):
                bc = ps_bc.tile([128, B], fp32)
                nc.tensor.matmul(bc, sel_sb[:, j, :], rS,
                                 start=True, stop=True)
                nc.vector.tensor_mul(att[:, j, :], u_sb[:, j, :], bc)

    nc.compile()
    return nc


def _host_prep(x, weight_att, weight_input, weight_e):
    """Build per-core input maps (host-side layout prep)."""
    xT = np.ascontiguousarray(x.transpose(2, 1, 0))  # [C, T, B]

    in_maps = []
    for g in range(NCORES):
        h0 = g * HL
        wi = np.zeros((C, NPAIR, 128), np.float32)
        wa = np.zeros((128, NPAIR, 128), np.float32)
        we = np.zeros((128, NPAIR, 128), np.float32)
        on = np.zeros((128, NPAIR, HL), np.float32)
        sel = np.zeros((HL, NPAIR, 128), np.float32)
        for j in range(NPAIR):
            ha, hb = h0 + 2 * j, h0 + 2 * j + 1
            # lhsT[k, m] = W[h, m, k]
            wi[:, j, 0:C] = weight_input[ha].T
            wi[:, j, C:128] = weight_input[hb].T
            wa[0:C, j, 0:C] = weight_att[ha].T
            wa[C:128, j, C:128] = weight_att[hb].T
            we[0:C, j, 0:C] = weight_e[ha].T
            we[C:128, j, C:128] = weight_e[hb].T
            on[0:C, j, 2 * j] = 1.0
            on[C:128, j, 2 * j + 1] = 1.0
            sel[2 * j, j, 0:C] = 1.0
            sel[2 * j + 1, j, C:128] = 1.0
        in_maps.append({
            "xT": xT, "wi": wi, "wa": wa, "we": we,
            "ones16": on, "sel": sel,
        })
    return in_maps


def run(x, weight_att, weight_input, weight_e, t_steps=T, trace=False):
    from concourse.bass_utils import run_bass_kernel_spmd

    nc = _build_nc(t_steps)
    in_maps = _host_prep(x, weight_att, weight_input, weight_e)
    if t_steps != T:
        for m in in_maps:
            m["xT"] = np.ascontiguousarray(m["xT"][:, :t_steps, :])
    res = run_bass_kernel_spmd(nc, in_maps, list(range(NCORES)), trace=trace)

    # results[g]["out"]: [t_steps, HL, B] -> out[b, t, g*HL + i]
    out = np.empty((B, t_steps, H), np.float32)
    for g in range(NCORES):
        og = res.results[g]["out"]
        out[:, :, g * HL:(g + 1) * HL] = og.transpose(2, 0, 1)
    return out, res


def kernel(x, weight_att, weight_input, weight_e):
    out, _ = run(x, weight_att, weight_input, weight_e)
    return out
